# revision 30
# baseline (speedup 1.0000x reference)
"""Trainium2 Bass kernel for nn_CrossAttentionPositionBridge.

Contract: kernel(**inputs) takes FULL unsharded inputs (as produced by
setup_inputs) and returns the FULL (4, 4096, 1024) float32 output.

Strategy (V4: bf16 value path + fp8-DoubleRow score path):
  - Each of the 4 rows is split at the first patch boundary >= 2048 into two
    chunks -> 8 chunks, one per NeuronCore (P=2176 padded positions).
  - Ragged segment ops are matmuls against host-uploaded selection matrices.
    selT[pos,np] (bf16) carries invcnt[np] in nonzero entries: the softmax
    denominator computed against it yields invd' = cnt/denom, which exactly
    cancels the scaling in the attn*v scatter; sel in fp8 (0/1) feeds the
    qmean and q-gather DoubleRow matmuls.
  - Score path (qmean, q-proj, q-gather, k-proj) runs fp8 e4m3 with
    DoubleRow perf mode (split-half packing: pair axis = explicit AP dim
    with Num=2, stride%16==0; host pre-scales x by 2/16 and weights by 64,
    rescales via the exp scale and a per-partition invcnt scale at the q8
    cast).  Value path (v, attn*v scatter, o2, out-gather) stays bf16;
    f32 accumulation in PSUM throughout.
  - decode stage folded on host: o2 = patch_heads @ (Wo2 @ Wv2 @ Wo).T.
  - Engines: PE matmuls; DVE score dot + half the casts; ACT exp + other
    half; SP DMA ring streams inputs in consumption order, ACT ring carries
    sel tables/wfull/output; PSUM tiles span 2 banks in P4/P5 so one
    cast+DMA covers both halves.
"""

import numpy as np
import ml_dtypes

import concourse.bass as bass
import concourse.mybir as mybir
import concourse.tile as tile
from concourse import bacc, bass_utils
from concourse.bass import ts

B, S, D, H = 4, 4096, 1024, 16
HD = D // H
P = 2176           # padded chunk length
TB = P // 128      # 17 position blocks
NP = 384           # padded patch count
NB = NP // 128     # 3 patch blocks
DC = D // 128      # 8 feature chunks
N_CORES = 8

F32 = mybir.dt.float32
BF16 = mybir.dt.bfloat16
FP8 = mybir.dt.float8e4
DR = mybir.MatmulPerfMode.DoubleRow
TP = 9             # pos-pair blocks for fp8 qmean (padded to 2304)
P2PAD = 2 * 128 * TP

_PROG_CACHE = {}


def _build_body(nc, tc, aps, flags):
    """Emit the per-core kernel body into the TileContext."""
    from contextlib import ExitStack

    xT, selT, selN = aps["xT"], aps["selT"], aps["selN"]
    x8, sel8, xt8, wk8, invc = (aps["x8"], aps["sel8"], aps["xt8"],
                                aps["wk8"], aps["invc"])
    selN8a, selN8b, wq8 = aps["selN8a"], aps["selN8b"], aps["wq8"]
    wvT, wfullT = aps["wvT"], aps["wfullT"]
    out = aps["out"]

    xT_r = xT.rearrange("(dc p) t -> p dc t", p=128)
    xs8_r = x8.rearrange("(tp q) f -> q tp f", q=128)
    xt8_r = xt8.rearrange("p (tb f) -> p tb f", f=8 * 128)
    selT_r = selT.rearrange("(tb p) n -> p tb n", p=128)
    selN_r = selN.rearrange("(nb p) (tb q) -> p nb tb q", p=128, q=128)
    out_r = out.rearrange("(tb p) d -> p tb d", p=128)
    wvT_r = wvT.rearrange("(dc p) d -> p dc d", p=128)
    invc_r = invc.rearrange("(nb p) -> p nb", p=128)
    wfullT_r = wfullT.rearrange("(dc p) d -> p dc d", p=128)

    MUL = mybir.AluOpType.mult

    def cast(i, dst, src):
        # alternate PSUM->SBUF staging casts between ACT and DVE
        if i % 2 == 0:
            nc.scalar.copy(dst, src)
        else:
            nc.vector.tensor_copy(dst, src)

    with ExitStack() as ctx:
        # ---- long-lived pools -------------------------------------------
        perm = ctx.enter_context(tc.tile_pool(name="perm", bufs=1))
        selT_sb = perm.tile([128, TB, NP], BF16)
        selN_sb = perm.tile([128, NB, TB, 128], BF16)
        qp_sb = perm.tile([128, TB, D], BF16)     # gathered per-pos query
        v_sb = perm.tile([128, TB, D], BF16)
        p_sb = perm.tile([128, TB, H], BF16)      # exp(score)
        attn_sb = perm.tile([128, TB, H], BF16)   # p * cnt/denom
        invd_sb = perm.tile([128, NB, H], BF16)
        upw_sb = perm.tile([128, DC, NP], BF16)
        wfull_sb = perm.tile([128, DC, D], BF16)
        invc_sb = perm.tile([128, NB], F32)
        selN8a_sb = perm.tile([128, 2, TB, 128], FP8)
        selN8b_sb = perm.tile([128, TB, 128], FP8)

        with ExitStack() as ctxw:
            pw = ctxw.enter_context(tc.tile_pool(name="pw", bufs=1))
            wk8_sb = pw.tile([128, 4, 2, D], FP8)
            wk8_flat = wk8_sb[:].rearrange("p a j n -> p (a j n)")
            wq8_sb = pw.tile([128, 4, 2, D], FP8)
            wq8_flat = wq8_sb[:].rearrange("p a j n -> p (a j n)")
            wv_sb = pw.tile([128, DC, D], BF16)
            pxt = ctxw.enter_context(tc.tile_pool(name="pxt", bufs=6))

            # ================= P1: qmeanT (sel carries invcnt) ===========
            with ExitStack() as ctx1:
                pqm = ctx1.enter_context(tc.tile_pool(name="pqm", bufs=1))
                qmT_sb = pqm.tile([128, DC, NP], FP8)
                q_sb = pqm.tile([128, NB, D], FP8)
                with tc.tile_pool(name="xs", bufs=3) as xs, \
                     tc.tile_pool(name="ps1", bufs=1, space="PSUM") as ps1:
                    qm_ps = [ps1.tile([128, NP], F32, tag=f"qm{db}",
                                      name=f"qm_ps{db}")
                             for db in range(DC)]
                    for tp in range(TP):
                        xs8_t = xs.tile([128, 2 * NP + 2 * D], FP8, tag="x8")
                        nc.sync.dma_start(xs8_t[:], xs8_r[:, tp, :])
                        s8_v = xs8_t[:, bass.ds(0, 2 * NP)].rearrange(
                            "p (j n) -> p j n", j=2)
                        x8_v = xs8_t[:, bass.ds(2 * NP, 2 * D)].rearrange(
                            "p (j f) -> p j f", j=2)
                        for db in range(DC):
                            nc.tensor.matmul(
                                qm_ps[db][:], x8_v[:, :, ts(db, 128)],
                                s8_v[:], perf_mode=DR,
                                start=(tp == 0), stop=(tp == TP - 1))
                    # SP ring, consumption order: wq8 (P1b), selN8 (P1c),
                    # wk8/wv (P2), then the xt8/xT stream.  selT (P2 denom),
                    # selN (P3a/P5) and wfull (P4) ride the ACT ring, gated.
                    nc.sync.dma_start(wq8_flat, wq8[:])
                    nc.sync.dma_start(
                        selN8a_sb[:].rearrange("p j t c -> p (j t c)"),
                        selN8a[:])
                    nc.sync.dma_start(
                        selN8b_sb[:].rearrange("p t c -> p (t c)"),
                        selN8b[:])
                    nc.sync.dma_start(wk8_flat, wk8[:])
                    nc.sync.dma_start(wv_sb[:], wvT_r[:])
                    xt_t = []
                    xt8_t = []
                    for tb in range(TB):
                        t8 = pxt.tile([128, 4, 2, 128], FP8, tag="xt8",
                                      name=f"xt8{tb}")
                        nc.sync.dma_start(
                            t8[:].rearrange("p a j c -> p (a j c)"),
                            xt8_r[:, tb, :])
                        xt8_t.append(t8)
                        t = pxt.tile([128, DC, 128], BF16, tag="xt",
                                     name=f"xt{tb}")
                        nc.sync.dma_start(t[:], xT_r[:, :, ts(tb, 128)])
                        xt_t.append(t)
                    # qmT8 holds 16*patch-sum; invcnt applied at q8 cast
                    for db in range(DC):
                        cast(db, qmT_sb[:, db, :], qm_ps[db][:])

                # ===== P1b: q8 = (16*invcnt/1024)*(qmT8 @ wq8), fp8-DR ====
                nc.scalar.dma_start(invc_sb[:], invc_r[:])
                with tc.tile_pool(name="ps2", bufs=4, space="PSUM") as ps2:
                    for nb in range(NB):
                        for hf in range(2):
                            q_ps = ps2.tile([128, 512], F32, tag="q")
                            for c4 in range(4):
                                nc.tensor.matmul(
                                    q_ps[:],
                                    qmT_sb[:, bass.ds(2 * c4, 2),
                                           ts(nb, 128)],
                                    wq8_sb[:, c4, :, bass.ds(hf * 512, 512)],
                                    perf_mode=DR,
                                    start=(c4 == 0), stop=(c4 == 3))
                            if (nb * 2 + hf) % 2 == 0:
                                nc.scalar.activation(
                                    q_sb[:, nb, ts(hf, 512)], q_ps[:],
                                    mybir.ActivationFunctionType.Copy,
                                    scale=invc_sb[:, nb, None])
                            else:
                                nc.vector.tensor_scalar_mul(
                                    q_sb[:, nb, ts(hf, 512)], q_ps[:],
                                    invc_sb[:, nb, None])

                    # ===== P1c: q_pos = selN8.T @ q8 (DR + tail chunk) ====
                    for tb in range(TB):
                        for hf in range(2):
                            qp_ps = ps2.tile([128, 512], F32, tag="qp")
                            nc.tensor.matmul(
                                qp_ps[:], selN8a_sb[:, :, tb, :],
                                q_sb[:, bass.ds(0, 2), ts(hf, 512)],
                                perf_mode=DR, start=True, stop=False)
                            nc.tensor.matmul(
                                qp_ps[:], selN8b_sb[:, tb, :],
                                q_sb[:, 2, ts(hf, 512)],
                                start=False, stop=True)
                            cast(tb * 2 + hf, qp_sb[:, tb, ts(hf, 512)],
                                 qp_ps[:])

            # selT (P2 denom), selN (P3a/P5), wfull (P4) on the ACT ring,
            # emitted after P1's ACT work so they can't block the casts
            for tb in range(TB):
                nc.scalar.dma_start(selT_sb[:, tb, :], selT_r[:, tb, :])
            nc.scalar.dma_start(selN_sb[:], selN_r[:])
            nc.scalar.dma_start(wfull_sb[:], wfullT_r[:])

            # ============ P2: k, v, scores, denominator ==================
            with tc.tile_pool(name="zs", bufs=3) as zs, \
                 tc.tile_pool(name="ps3", bufs=2, space="PSUM") as ps3, \
                 tc.tile_pool(name="psdn", bufs=1, space="PSUM") as psdn:
                dn_ps = [psdn.tile([128, H], F32, tag=f"dn{nb}",
                                   name=f"dn_ps{nb}") for nb in range(NB)]
                for tb in range(TB):
                    for hf in range(2):
                        k_ps = ps3.tile([128, 512], F32, tag="k")
                        for dc4 in range(4):
                            nc.tensor.matmul(
                                k_ps[:], xt8_t[tb][:, dc4, :, :],
                                wk8_sb[:, dc4, :, bass.ds(hf * 512, 512)],
                                perf_mode=DR,
                                start=(dc4 == 0), stop=(dc4 == 3))
                        v_ps = ps3.tile([128, 512], F32, tag="v")
                        for db in range(DC):
                            nc.tensor.matmul(
                                v_ps[:], xt_t[tb][:, db, :],
                                wv_sb[:, db, ts(hf, 512)],
                                start=(db == 0), stop=(db == DC - 1))
                        # score: z = k * q_pos (DVE reads one PSUM operand)
                        z_t = zs.tile([128, 512], F32, tag="z")
                        nc.vector.tensor_tensor(
                            z_t[:], k_ps[:], qp_sb[:, tb, ts(hf, 512)], MUL)
                        sc_t = zs.tile([128, 8], F32, tag="sc")
                        nc.vector.tensor_reduce(
                            sc_t[:],
                            z_t[:].rearrange("p (h e) -> p h e", e=HD),
                            mybir.AxisListType.X, mybir.AluOpType.add)
                        nc.scalar.activation(
                            p_sb[:, tb, bass.ds(hf * 8, 8)], sc_t[:],
                            mybir.ActivationFunctionType.Exp,
                            scale=1.0 / (float(HD) ** 0.5 * 1024.0 * 16.0))
                        nc.scalar.copy(v_sb[:, tb, ts(hf, 512)], v_ps[:])
                    # denominator (against scaled selT -> invd' = cnt/denom)
                    for nb in range(NB):
                        nc.tensor.matmul(
                            dn_ps[nb][:], selT_sb[:, tb, ts(nb, 128)],
                            p_sb[:, tb, :],
                            start=(tb == 0), stop=(tb == TB - 1))

                # invd' = 1/(dn + 1e-30)
                for nb in range(NB):
                    dn_t = zs.tile([128, H], F32, tag="dn")
                    nc.vector.tensor_scalar_add(dn_t[:], dn_ps[nb][:], 1e-30)
                    rec_t = zs.tile([128, H], F32, tag="rec")
                    nc.vector.reciprocal(rec_t[:], dn_t[:])
                    nc.vector.tensor_copy(invd_sb[:, nb, :], rec_t[:])

        # ================= P3a: attn = p * invd'[seg] =====================
        with tc.tile_pool(name="ps3a", bufs=4, space="PSUM") as ps3a:
            for tb in range(TB):
                idp_ps = ps3a.tile([128, H], F32, tag="idp")
                for nb in range(NB):
                    nc.tensor.matmul(
                        idp_ps[:], selN_sb[:, nb, tb, :],
                        invd_sb[:, nb, :],
                        start=(nb == 0), stop=(nb == NB - 1))
                nc.vector.tensor_tensor(attn_sb[:, tb, :], p_sb[:, tb, :],
                                        idp_ps[:], MUL)

        # ========== P3b: w = attn*v ; upw = patch_headsT ==================
        if True:
            with tc.tile_pool(name="vs", bufs=3) as vs, \
                 tc.tile_pool(name="ps3b", bufs=1, space="PSUM") as ps3b:
                upw_ps = [ps3b.tile([128, NP], F32, tag=f"up{db}",
                                    name=f"upw_ps{db}") for db in range(DC)]
                for tb in range(TB):
                    w_t = vs.tile([128, H, HD], BF16, tag="w")
                    nc.vector.tensor_tensor(
                        w_t[:],
                        v_sb[:, tb, :].rearrange("p (h e) -> p h e", e=HD),
                        attn_sb[:, tb, :, None].to_broadcast([128, H, HD]),
                        MUL)
                    w_f = w_t[:].rearrange("p h e -> p (h e)")
                    for db in range(DC):
                        nc.tensor.matmul(
                            upw_ps[db][:], w_f[:, ts(db, 128)],
                            selT_sb[:, tb, :],
                            start=(tb == 0), stop=(tb == TB - 1))
                for db in range(DC):
                    cast(db, upw_sb[:, db, :], upw_ps[db][:])

        # ========= P4: o2 = patch_heads @ WfullT ==========================
        # ========= P5: out = selN.T @ o2 (bf16 output) ====================
        with tc.tile_pool(name="p4", bufs=1) as p4, \
             tc.tile_pool(name="ps4", bufs=2, space="PSUM") as ps4, \
             tc.tile_pool(name="oc", bufs=6) as oc:
            o2_sb = p4.tile([128, NB, D], BF16)
            for nb in range(NB):
                o2_ps = ps4.tile([128, 2, 512], F32, tag="o2")
                for hf in range(2):
                    for dc in range(DC):
                        nc.tensor.matmul(
                            o2_ps[:, hf, :], upw_sb[:, dc, ts(nb, 128)],
                            wfull_sb[:, dc, ts(hf, 512)],
                            start=(dc == 0), stop=(dc == DC - 1))
                cast(nb, o2_sb[:, nb, :],
                     o2_ps[:].rearrange("p j n -> p (j n)"))

            for tb in range(TB):
                o_ps = ps4.tile([128, 2, 512], F32, tag="o")
                for hf in range(2):
                    for nb in range(NB):
                        nc.tensor.matmul(
                            o_ps[:, hf, :], selN_sb[:, nb, tb, :],
                            o2_sb[:, nb, ts(hf, 512)],
                            start=(nb == 0), stop=(nb == NB - 1))
                oc_t = oc.tile([128, D], BF16, tag="oc")
                cast(tb, oc_t[:], o_ps[:].rearrange("p j n -> p (j n)"))
                nc.scalar.dma_start(out_r[:, tb, :], oc_t[:])


def _build_program(flags, loop_reps=None):
    nc = bacc.Bacc("TRN2", target_bir_lowering=False, debug=False)
    aps = {}
    aps["xT"] = nc.dram_tensor("xT", [D, P], BF16, kind="ExternalInput").ap()
    aps["selT"] = nc.dram_tensor("selT", [P, NP], BF16,
                                 kind="ExternalInput").ap()
    aps["selN"] = nc.dram_tensor("selN", [NP, P], BF16,
                                 kind="ExternalInput").ap()
    aps["x8"] = nc.dram_tensor("x8", [TP * 128, 2 * NP + 2 * D], FP8,
                               kind="ExternalInput").ap()
    aps["sel8"] = nc.dram_tensor("sel8", [TP * 128, 2 * NP], FP8,
                                 kind="ExternalInput").ap()
    aps["xt8"] = nc.dram_tensor("xt8", [128, TB * 8 * 128], FP8,
                                kind="ExternalInput").ap()
    aps["wk8"] = nc.dram_tensor("wk8", [128, 8 * D], FP8,
                                kind="ExternalInput").ap()
    aps["invc"] = nc.dram_tensor("invc", [NP], F32,
                                 kind="ExternalInput").ap()
    aps["selN8a"] = nc.dram_tensor("selN8a", [128, 2 * P], FP8,
                                   kind="ExternalInput").ap()
    aps["selN8b"] = nc.dram_tensor("selN8b", [128, P], FP8,
                                   kind="ExternalInput").ap()
    aps["wq8"] = nc.dram_tensor("wq8", [128, 8 * D], FP8,
                                kind="ExternalInput").ap()
    for w in ("wvT", "wfullT"):
        aps[w] = nc.dram_tensor(w, [D, D], BF16, kind="ExternalInput").ap()
    if loop_reps is not None:
        # Timing build: the big output stays in internal DRAM so the host
        # only ships a tiny donated zero buffer per timed call.
        aps["out"] = nc.dram_tensor("out_scratch", [P, D], BF16).ap()
        dummy = nc.dram_tensor("out", [1, 1], F32, kind="ExternalOutput").ap()
    else:
        aps["out"] = nc.dram_tensor("out", [P, D], BF16,
                                    kind="ExternalOutput").ap()

    with tile.TileContext(nc) as tc:
        if loop_reps is not None:
            with tc.For_i(0, loop_reps, 1):
                _build_body(nc, tc, aps, flags)
            with tc.tile_pool(name="dum", bufs=1) as dum:
                d_t = dum.tile([1, 1], F32)
                nc.vector.memset(d_t[:], 0.0)
                nc.sync.dma_start(dummy[:], d_t[:])
        else:
            _build_body(nc, tc, aps, flags)
    nc.compile()
    return nc


def get_program(flags=None, loop_reps=None):
    if flags is None:
        flags = {}
    key = (tuple(sorted(flags.items())), loop_reps)
    if key not in _PROG_CACHE:
        _PROG_CACHE[key] = _build_program(flags, loop_reps)
    return _PROG_CACHE[key]


def _make_shards(patch_boundaries):
    pb = np.asarray(patch_boundaries)
    shards = []
    for b in range(pb.shape[0]):
        bnd = (pb[b] != 0).astype(np.int64)
        pid = np.cumsum(bnd) - bnd[0]
        bpos = np.nonzero(bnd)[0]
        cand = bpos[bpos >= S // 2]
        split = int(cand[0]) if len(cand) else S
        for (t0, t1) in ((0, split), (split, S)):
            L = t1 - t0
            assert L <= P, f"chunk length {L} exceeds padded size {P}"
            pad_pid = np.full(P, NP - 1, np.int64)
            if L:
                lpid = pid[t0:t1] - pid[t0]
                assert lpid[-1] + 1 <= NP - 1, "too many patches in chunk"
                pad_pid[:L] = lpid
            cnt = np.bincount(pad_pid[:L], minlength=NP).astype(np.float32)
            invcnt = np.zeros(NP, np.float32)
            nz = cnt > 0
            invcnt[nz] = 1.0 / cnt[nz]
            invcnt[NP - 1] = 0.0
            shards.append(dict(row=b, t0=t0, L=L, pid=pad_pid, invcnt=invcnt))
    return shards


def prepare_in_maps(byte_repr, Wq, bq, Wk, bk, Wv, bv, Wo, bo, Wv2, bv2,
                    Wo2, bo2, patch_boundaries):
    """Host-side sharding/marshalling: returns (shards, in_maps, flags)."""
    bf16 = ml_dtypes.bfloat16
    byte_repr = np.asarray(byte_repr, np.float32)
    shards = _make_shards(patch_boundaries)
    Wo = np.asarray(Wo, np.float64)
    Wv2 = np.asarray(Wv2, np.float64)
    Wo2 = np.asarray(Wo2, np.float64)
    wfull = Wo2 @ (Wv2 @ Wo)
    bfull = (Wo2 @ (Wv2 @ np.asarray(bo, np.float64)
                    + np.asarray(bv2, np.float64))
             + np.asarray(bo2, np.float64))
    assert not np.any(np.asarray(bq)) and not np.any(np.asarray(bk)), \
        "nonzero attention biases unsupported"
    assert not np.any(np.asarray(bv)) and not np.any(bfull), \
        "nonzero value/output biases unsupported"
    flags = {}
    fp8 = ml_dtypes.float8_e4m3
    wvT = np.ascontiguousarray(np.asarray(Wv, np.float32).T).astype(bf16)
    wqT64 = np.asarray(Wq, np.float32).T * 64.0
    wq8 = np.ascontiguousarray(
        wqT64.reshape(4, 2, 128, D).transpose(2, 0, 1, 3).reshape(128, 8 * D)
    ).astype(fp8)
    wfullT = np.ascontiguousarray(wfull.T.astype(np.float32)).astype(bf16)
    # wk8: [d', (dc4, j, n)] = 64*Wk[n, dc4*256+j*128+d']  (split-half DR)
    wkT64 = np.asarray(Wk, np.float32).T * 64.0     # [d, n]
    wk8 = np.ascontiguousarray(
        wkT64.reshape(4, 2, 128, D).transpose(2, 0, 1, 3).reshape(128, 8 * D)
    ).astype(fp8)

    in_maps = []
    iota_np = np.arange(NP, dtype=np.int64)
    for sh in shards:
        xc = np.zeros((P, D), np.float32)
        if sh["L"]:
            xc[:sh["L"]] = byte_repr[sh["row"], sh["t0"]:sh["t0"] + sh["L"]]
        sel01 = (sh["pid"][:, None] == iota_np[None, :])
        selT = (sel01 * sh["invcnt"][None, :].astype(np.float64)
                ).astype(np.float32).astype(bf16)
        # qmean operand: 2x (patch sums must stay inside fp8's +-240)
        xpad = np.zeros((2 * 128 * TP, D), np.float32)
        xpad[:P] = xc * 2.0
        x8 = np.ascontiguousarray(
            xpad.reshape(TP, 2, 128, D).transpose(0, 2, 1, 3)
            .reshape(TP * 128, 2 * D)).astype(fp8)
        spad = np.zeros((2 * 128 * TP, NP), np.float32)
        spad[:P] = sel01.astype(np.float32)
        sel8 = np.ascontiguousarray(
            spad.reshape(TP, 2, 128, NP).transpose(0, 2, 1, 3)
            .reshape(TP * 128, 2 * NP)).astype(fp8)
        xs8 = np.concatenate([sel8, x8], axis=1)
        # xt8: [d', (tb, dc4, j, p)] = 16*x[tb*128+p, dc4*256+j*128+d']
        xt8 = np.ascontiguousarray(
            (xc * 16.0).reshape(TB, 128, 4, 2, 128).transpose(4, 0, 2, 3, 1)
            .reshape(128, TB * 8 * 128)).astype(fp8)
        m = {
            "xT": np.ascontiguousarray(xc.T).astype(bf16),
            "selT": selT,
            "selN": np.ascontiguousarray(sel01.T.astype(bf16)),
            "x8": xs8, "sel8": sel8, "xt8": xt8, "wk8": wk8, "wq8": wq8,
            "selN8a": np.ascontiguousarray(
                sel01[:, :256].T.reshape(2, 128, P).transpose(1, 0, 2)
                .reshape(128, 2 * P).astype(np.float32)).astype(fp8),
            "selN8b": np.ascontiguousarray(
                sel01[:, 256:].T.astype(np.float32)).astype(fp8),
            "invc": (sh["invcnt"] / 8.0).astype(np.float32),
            "wvT": wvT, "wfullT": wfullT,
        }
        in_maps.append(m)
    return shards, in_maps, flags


def kernel(byte_repr, Wq, bq, Wk, bk, Wv, bv, Wo, bo, Wv2, bv2, Wo2, bo2,
           patch_boundaries):
    shards, in_maps, flags = prepare_in_maps(
        byte_repr, Wq, bq, Wk, bk, Wv, bv, Wo, bo, Wv2, bv2, Wo2, bo2,
        patch_boundaries)
    nc = get_program(flags)
    res = bass_utils.run_bass_kernel_spmd(nc, in_maps, list(range(N_CORES)))
    out = np.zeros((B, S, D), np.float32)
    for sh, r in zip(shards, res.results):
        if sh["L"]:
            out[sh["row"], sh["t0"]:sh["t0"] + sh["L"]] = (
                r["out"][:sh["L"]].astype(np.float32))
    return out
